# revision 20
# baseline (speedup 1.0000x reference)
# Bidirectional LSTM (B=512, T=256, E=256, U=512) + MLP + softmax(V=10000)
# on 8 trn2 NeuronCores.
#
# Distribution (v2, collective-free): each core owns 64 batch rows and runs
# BOTH directions for those rows. Within a core, partitions 0-63 carry the
# forward chain and partitions 64-127 the backward chain (time-reversed
# token stream, interleaved in the gather index table). All matmuls are
# emitted as column-tiled pairs (tile_position (0,0) / (0,64)) so the fw and
# bw halves use disjoint PE column groups and can run concurrently. The MLP
# head needs h_fw[r] (+) h_bw[r] of the same row r, which now live on the
# same core: transpose the partial product, add the partition halves in the
# free dimension, relu, then h @ W2 + softmax. No cross-core communication
# (the pairwise AllReduce of v1 cost ~31 ms of wall overhead per call).
#
# Per step t (one core):
#   gates[128,2048] (PSUM, fp32): x_t @ Wx + h_{t-1} @ Wh as lhsT.T @ rhs
#     with data transposed as stationary (xT via transposing dma_gather;
#     hT via per-step PE transpose) and bf16 weights streaming; fw rows use
#     Wx_fw/Wh_fw (cols 0-63 of the stationary), bw rows Wx_bw/Wh_bw.
#   i,f,o = sigmoid(gates[:,0:1536]); g = tanh(gates[:,1536:2048])  (ScalarE,
#     gate columns pre-permuted to [i f o g] on the host)
#   c = f*c + i*g (DVE, fp32 state);  h = o * tanh(c)  (bf16)
import os
import numpy as np
import ml_dtypes

B, T, E, U, V = 512, 256, 256, 512, 10000
G4 = 4 * U
NCORES = 8
RC = 64               # batch rows per core (each run in both directions)
PC = 2 * RC           # partition rows per core = 64 fw + 64 bw
NK_X = E // 128       # 2 contraction tiles for x
NK_H = U // 128       # 4 contraction tiles for h
NBW = 512             # matmul n-block width (one PSUM bank)
NB = G4 // NBW        # 4 n-blocks
TOK = PC * T          # 32768 tokens gathered per core
T_STEPS = int(os.environ.get("KERNEL_T", T))
CHUNK_STEPS = 4       # 512 tokens per dma_gather (>512 idxs crashes SWDGE)
CHUNK_TOK = PC * CHUNK_STEPS
NCHUNK = (T_STEPS + CHUNK_STEPS - 1) // CHUNK_STEPS
VCH = 500             # logits chunk width
NVCH = V // VCH

_prog_cache = {}


def _build_program(with_gate_bias: bool, with_b2: bool):
    import concourse.bass as bass
    import concourse.mybir as mybir
    import concourse.tile as tile
    from concourse import bacc
    from concourse.masks import make_identity
    from contextlib import ExitStack

    f32 = mybir.dt.float32
    bf16 = mybir.dt.bfloat16
    i16 = mybir.dt.int16
    AF = mybir.ActivationFunctionType

    nc = bacc.Bacc("TRN2", debug=False, enable_asserts=False, num_devices=NCORES)

    emb_d = nc.dram_tensor("emb16", [V, E], bf16, kind="ExternalInput").ap()
    idx_d = nc.dram_tensor("idx16", [128, TOK // 16], i16, kind="ExternalInput").ap()
    # dir-major: [fw k-tiles..., bw k-tiles...]
    wx_d = nc.dram_tensor("wx", [2 * NK_X, 128, G4], bf16, kind="ExternalInput").ap()
    wh_d = nc.dram_tensor("wh", [2 * NK_H, 128, G4], bf16, kind="ExternalInput").ap()
    w1_d = nc.dram_tensor("w1h", [2 * NK_H, 128, 64], bf16, kind="ExternalInput").ap()
    w2_d = nc.dram_tensor("w2", [64, V], bf16, kind="ExternalInput").ap()
    b1_d = nc.dram_tensor("b1p", [64, 1], f32, kind="ExternalInput").ap()
    if with_gate_bias:
        bg_d = nc.dram_tensor("bgbc", [128, G4], f32, kind="ExternalInput").ap()
    if with_b2:
        b2_d = nc.dram_tensor("b2bc", [64, V], f32, kind="ExternalInput").ap()
    out_d = nc.dram_tensor("out", [RC, V], f32, kind="ExternalOutput").ap()

    with tile.TileContext(nc) as tc, ExitStack() as ctx:
        const = ctx.enter_context(tc.tile_pool(name="const", bufs=1))
        gpool = ctx.enter_context(tc.tile_pool(name="gather", bufs=3))
        work = ctx.enter_context(tc.tile_pool(name="work", bufs=2))
        psum = ctx.enter_context(tc.tile_pool(name="psum", bufs=1, space="PSUM"))

        # token indices first so the first gather isn't queued behind 7MB
        # of weight DMAs
        idx_sb = const.tile([128, TOK // 16], i16)
        nc.sync.dma_start(idx_sb[:], idx_d[:])
        wx_sb = const.tile([128, 2 * NK_X, G4], bf16)
        for k in range(2 * NK_X):
            nc.sync.dma_start(wx_sb[:, k, :], wx_d[k])
        wh_sb = const.tile([128, 2 * NK_H, G4], bf16)
        for k in range(2 * NK_H):
            nc.sync.dma_start(wh_sb[:, k, :], wh_d[k])
        w1_sb = const.tile([128, 2 * NK_H, 64], bf16)
        for k in range(2 * NK_H):
            nc.sync.dma_start(w1_sb[:, k, :], w1_d[k])
        w2_sb = const.tile([64, V], bf16)
        nc.sync.dma_start(w2_sb[:], w2_d[:])
        b1_sb = const.tile([64, 1], f32)
        nc.sync.dma_start(b1_sb[:], b1_d[:])
        if with_gate_bias:
            bg_sb = const.tile([128, G4], f32)
            nc.sync.dma_start(bg_sb[:], bg_d[:])
            # DVE pre-copy so downstream tensor_tensor ops have a same-engine
            # dep (walrus TT format has a single sync-wait slot).
            bgc = const.tile([128, G4], f32)
            nc.vector.tensor_copy(bgc[:], bg_sb[:])
        if with_b2:
            b2_sb = const.tile([64, V], f32)
            nc.sync.dma_start(b2_sb[:], b2_d[:])
        ident = const.tile([128, 128], bf16)
        make_identity(nc, ident[:])
        # c state kept as two half tiles so the a-half update chain never
        # waits on the b-half (tile-granularity dep tracking)
        U2 = U // 2
        c_a = const.tile([128, U2], f32)
        c_b = const.tile([128, U2], f32)

        xg_tiles = {}

        def issue_gather(ci):
            xg = gpool.tile(
                [128, NK_X, CHUNK_TOK], bf16, tag="xg", name=f"xg{ci}"
            )
            nc.gpsimd.dma_gather(
                xg[:],
                emb_d[:],
                idx_sb[:, ci * (CHUNK_TOK // 16):(ci + 1) * (CHUNK_TOK // 16)],
                CHUNK_TOK,
                CHUNK_TOK,
                E,
                transpose=True,
            )
            xg_tiles[ci] = xg

        issue_gather(0)

        def mm_pair(out, lhsT_fw, rhs_fw, lhsT_bw, rhs_bw, start, stop):
            nc.tensor.matmul(
                out[0:RC, :], lhsT=lhsT_fw, rhs=rhs_fw,
                start=start, stop=stop, tile_position=(0, 0),
            )
            nc.tensor.matmul(
                out[RC:PC, :], lhsT=lhsT_bw, rhs=rhs_bw,
                start=start, stop=stop, tile_position=(0, RC),
            )

        hT_prev = None
        for t in range(T_STEPS):
            ci = t // CHUNK_STEPS
            w = t % CHUNK_STEPS
            if w == 1 and ci + 1 < NCHUNK:
                issue_gather(ci + 1)
            xg = xg_tiles[ci]

            # separate PSUM tiles per activation-read region (deps are
            # tracked per tile, so one tile per reader avoids false WARs
            # serializing the matmul stream)
            gif = psum.tile([128, 2 * U], f32, tag="gif", name=f"gif{t}")
            ggp = psum.tile([128, U], f32, tag="ggp", name=f"ggp{t}")
            gop = psum.tile([128, U], f32, tag="gop", name=f"gop{t}")
            n_kt = NK_X + (NK_H if hT_prev is not None else 0)

            def emit_block(out, c0, cw):
                # full contraction for one gate column range [c0, c0+cw)
                # (own PSUM bank, own accumulation group)
                ki = 0
                for k in range(NK_X):
                    mm_pair(
                        out,
                        xg[:, k, w * PC:w * PC + RC],
                        wx_sb[:, k, c0:c0 + cw],
                        xg[:, k, w * PC + RC:(w + 1) * PC],
                        wx_sb[:, NK_X + k, c0:c0 + cw],
                        start=(ki == 0),
                        stop=(ki == n_kt - 1),
                    )
                    ki += 1
                if hT_prev is not None:
                    for k in range(NK_H):
                        hT = hT_prev[k // 2]
                        kl = k % 2
                        mm_pair(
                            out,
                            hT[:, kl * 128:kl * 128 + RC],
                            wh_sb[:, k, c0:c0 + cw],
                            hT[:, kl * 128 + RC:(kl + 1) * 128],
                            wh_sb[:, NK_H + k, c0:c0 + cw],
                            start=(ki == 0),
                            stop=(ki == n_kt - 1),
                        )
                        ki += 1

            # col layout is [i f o g]; process i,f first (one sigmoid),
            # then g, then o so the c-update chain starts early
            emit_block(gif[:, 0:U], 0, U)
            emit_block(gif[:, U:2 * U], U, U)
            if with_gate_bias:
                nc.vector.tensor_add(gif[:], gif[:], bgc[:, 0:2 * U])
            ifo = work.tile([128, 2 * U], bf16, tag="ifo", name=f"ifo{t}")
            nc.scalar.activation(ifo[:], gif[:], AF.Sigmoid)
            emit_block(ggp[:], 3 * U, U)
            if with_gate_bias:
                nc.vector.tensor_add(ggp[:], ggp[:], bgc[:, 3 * U:G4])
            gga = work.tile([128, U2], bf16, tag="gga", name=f"gga{t}")
            nc.scalar.activation(gga[:], ggp[:, 0:U2], AF.Tanh)
            ggb = work.tile([128, U2], bf16, tag="ggb", name=f"ggb{t}")
            nc.scalar.activation(ggb[:], ggp[:, U2:U], AF.Tanh)
            emit_block(gop[:], 2 * U, U)
            if with_gate_bias:
                nc.vector.tensor_add(gop[:], gop[:], bgc[:, 2 * U:3 * U])
            oo = work.tile([128, U], bf16, tag="oo", name=f"oo{t}")
            nc.scalar.activation(oo[:], gop[:], AF.Sigmoid)

            # c/h updated in independent a/b halves (256 cols each) so the
            # a-half's hT lands early and next step's k0/k1 matmuls start
            # while the b-half still runs
            if t == 0:
                nc.vector.tensor_mul(c_a[:], ifo[:, 0:U2], gga[:])
                nc.vector.tensor_mul(c_b[:], ifo[:, U2:U], ggb[:])
            else:
                fca = work.tile([128, U2], f32, tag="fca", name=f"fca{t}")
                nc.vector.tensor_mul(fca[:], ifo[:, U:U + U2], c_a[:])
                ppa = work.tile([128, U2], bf16, tag="ppa", name=f"ppa{t}")
                nc.vector.tensor_mul(ppa[:], ifo[:, 0:U2], gga[:])
                nc.vector.tensor_add(c_a[:], fca[:], ppa[:])
                fcb = work.tile([128, U2], f32, tag="fcb", name=f"fcb{t}")
                nc.vector.tensor_mul(fcb[:], ifo[:, U + U2:2 * U], c_b[:])
                ppb = work.tile([128, U2], bf16, tag="ppb", name=f"ppb{t}")
                nc.vector.tensor_mul(ppb[:], ifo[:, U2:U], ggb[:])
                nc.vector.tensor_add(c_b[:], fcb[:], ppb[:])
            tcta = work.tile([128, U2], bf16, tag="tcta", name=f"tcta{t}")
            nc.scalar.activation(tcta[:], c_a[:], AF.Tanh)
            tctb = work.tile([128, U2], bf16, tag="tctb", name=f"tctb{t}")
            nc.scalar.activation(tctb[:], c_b[:], AF.Tanh)
            ha = work.tile([128, U2], bf16, tag="ha", name=f"ha{t}")
            nc.vector.tensor_mul(ha[:], oo[:, 0:U2], tcta[:])
            hb = work.tile([128, U2], bf16, tag="hb", name=f"hb{t}")
            nc.vector.tensor_mul(hb[:], oo[:, U2:U], tctb[:])

            trpa = psum.tile([128, U2], bf16, tag="trpa", bufs=2, name=f"trpa{t}")
            for k in range(2):
                nc.tensor.transpose(
                    trpa[:, k * 128:(k + 1) * 128],
                    ha[:, k * 128:(k + 1) * 128],
                    ident[:],
                )
            hTa = work.tile([128, U2], bf16, tag="hTa", name=f"hTa{t}")
            # ScalarE copy: DVE is busy with ha/hb here and ACT is free;
            # this copy gates next step's k0/k1 matmuls
            nc.scalar.copy(hTa[:], trpa[:])
            trpb = psum.tile([128, U2], bf16, tag="trpb", bufs=2, name=f"trpb{t}")
            for k in range(2):
                nc.tensor.transpose(
                    trpb[:, k * 128:(k + 1) * 128],
                    hb[:, k * 128:(k + 1) * 128],
                    ident[:],
                )
            hTb = work.tile([128, U2], bf16, tag="hTb", name=f"hTb{t}")
            nc.scalar.copy(hTb[:], trpb[:])
            hT_prev = (hTa, hTb)

        # ---- MLP head: P[r] = h_fw[r] @ W1a (rows 0-63) / h_bw[r] @ W1b
        # (rows 64-127); transpose, add the halves, relu(+b1) -> hidT.
        pps = psum.tile([128, 64], f32, tag="gif", name="pps")
        for k in range(NK_H):
            hTk = hT_prev[k // 2]
            kl = k % 2
            nc.tensor.matmul(
                pps[0:RC, :], lhsT=hTk[:, kl * 128:kl * 128 + RC],
                rhs=w1_sb[:, k, :],
                start=(k == 0), stop=(k == NK_H - 1), tile_position=(0, 0),
            )
            nc.tensor.matmul(
                pps[RC:PC, :], lhsT=hTk[:, kl * 128 + RC:(kl + 1) * 128],
                rhs=w1_sb[:, NK_H + k, :],
                start=(k == 0), stop=(k == NK_H - 1), tile_position=(0, RC),
            )
        p_sb = work.tile([128, 64], bf16, tag="p_sb", bufs=1)
        nc.vector.tensor_copy(p_sb[:], pps[:])
        ppt = psum.tile([64, 128], bf16, tag="trpa", bufs=2, name="ppt")
        nc.tensor.transpose(ppt[:], p_sb[:], ident[:])
        pt_sb = work.tile([64, 128], bf16, tag="pt_sb", bufs=1)
        nc.vector.tensor_copy(pt_sb[:], ppt[:])
        psum_h = work.tile([64, 64], bf16, tag="psum_h", bufs=1)
        nc.vector.tensor_add(psum_h[:], pt_sb[:, 0:RC], pt_sb[:, RC:PC])
        hidT = work.tile([64, 64], bf16, tag="hidT", bufs=1)
        nc.scalar.activation(hidT[:], psum_h[:], AF.Relu, bias=b1_sb[:])

        logits = work.tile([64, V], f32, tag="logits", bufs=1)
        for vc in range(NVCH):
            lp = psum.tile([64, VCH], f32, tag="trpa", bufs=2, name=f"lp{vc}")
            nc.tensor.matmul(
                lp[:],
                lhsT=hidT[:],
                rhs=w2_sb[:, vc * VCH:(vc + 1) * VCH],
                start=True,
                stop=True,
            )
            nc.vector.tensor_copy(logits[:, vc * VCH:(vc + 1) * VCH], lp[:])
        if with_b2:
            nc.vector.tensor_add(logits[:], logits[:], b2_sb[:])

        negmax = work.tile([64, 1], f32, tag="negmax", bufs=1)
        nc.vector.reduce_max(
            negmax[:], logits[:], axis=mybir.AxisListType.X, negate=True
        )
        exps = work.tile([64, V], bf16, tag="exps", bufs=1)
        sume = work.tile([64, 1], f32, tag="sume", bufs=1)
        nc.scalar.activation(
            exps[:], logits[:], AF.Exp, bias=negmax[:], accum_out=sume[:]
        )
        rcp = work.tile([64, 1], f32, tag="rcp", bufs=1)
        nc.vector.reciprocal(rcp[:], sume[:])
        nc.vector.tensor_scalar_mul(logits[:], exps[:], rcp[:])
        nc.sync.dma_start(out_d[:], logits[:])

    nc.finalize()
    return nc


def _get_program(with_gate_bias: bool, with_b2: bool):
    key = (with_gate_bias, with_b2, T_STEPS)
    if key not in _prog_cache:
        _prog_cache[key] = _build_program(with_gate_bias, with_b2)
    return _prog_cache[key]


# gate column permutation: reference order [i f g o] -> kernel order [i f o g]
_PERM = np.concatenate(
    [np.arange(0, U), np.arange(U, 2 * U), np.arange(3 * U, 4 * U),
     np.arange(2 * U, 3 * U)]
)


def _pack_w(Wx, Wh, b):
    bf = ml_dtypes.bfloat16
    wxp = np.ascontiguousarray(
        Wx[:, _PERM].reshape(NK_X, 128, G4).astype(bf)
    )
    whp = np.ascontiguousarray(
        Wh[:, _PERM].reshape(NK_H, 128, G4).astype(bf)
    )
    bp = np.ascontiguousarray(b[_PERM].astype(np.float32))
    return wxp, whp, bp


def _make_idx(tokens_tmajor_flat):
    # dma_gather reads index i from [i % 16, i // 16]; the 16-partition index
    # block must be replicated for each of the 8 gpsimd cores (128 partitions).
    wrapped = tokens_tmajor_flat.astype(np.int16).reshape(-1, 16).T
    return np.ascontiguousarray(np.tile(wrapped, (8, 1)))


def prepare(inputs):
    """Build (nc, in_maps) for the 8 cores from full unsharded inputs."""
    bf = ml_dtypes.bfloat16
    sentence = np.asarray(inputs["sentence"])
    emb = np.asarray(inputs["emb"], np.float32)
    Wx_fw = np.asarray(inputs["Wx_fw"], np.float32)
    Wh_fw = np.asarray(inputs["Wh_fw"], np.float32)
    b_fw = np.asarray(inputs["b_fw"], np.float32)
    Wx_bw = np.asarray(inputs["Wx_bw"], np.float32)
    Wh_bw = np.asarray(inputs["Wh_bw"], np.float32)
    b_bw = np.asarray(inputs["b_bw"], np.float32)
    W1 = np.asarray(inputs["W1"], np.float32)
    b1 = np.asarray(inputs["b1"], np.float32)
    W2 = np.asarray(inputs["W2"], np.float32)
    b2 = np.asarray(inputs["b2"], np.float32)

    with_gate_bias = bool(np.any(b_fw) or np.any(b_bw))
    with_b2 = bool(np.any(b2))
    nc = _get_program(with_gate_bias, with_b2)

    emb16 = np.ascontiguousarray(emb.astype(bf))
    wx_f, wh_f, bg_f = _pack_w(Wx_fw, Wh_fw, b_fw)
    wx_b, wh_b, bg_b = _pack_w(Wx_bw, Wh_bw, b_bw)
    wx = np.ascontiguousarray(np.concatenate([wx_f, wx_b], axis=0))
    wh = np.ascontiguousarray(np.concatenate([wh_f, wh_b], axis=0))
    w1f = W1[0:U].reshape(NK_H, 128, 64)
    w1b = W1[U:2 * U].reshape(NK_H, 128, 64)
    w1 = np.ascontiguousarray(
        np.concatenate([w1f, w1b], axis=0).astype(bf)
    )
    w2p = np.ascontiguousarray(W2.astype(bf))
    b1p = np.ascontiguousarray(b1.reshape(64, 1).astype(np.float32))

    in_maps = []
    for c in range(NCORES):
        rows = slice(RC * c, RC * c + RC)
        toks_fw = sentence[rows][:, :T]
        toks_bw = toks_fw[:, ::-1]
        # t-major, per step [64 fw tokens | 64 bw tokens]
        flat = np.ascontiguousarray(
            np.concatenate([toks_fw.T, toks_bw.T], axis=1)
        ).reshape(-1)
        m = {
            "emb16": emb16,
            "idx16": _make_idx(flat),
            "wx": wx,
            "wh": wh,
            "w1h": w1,
            "w2": w2p,
            "b1p": b1p,
        }
        if with_gate_bias:
            bg = np.concatenate(
                [np.broadcast_to(bg_f[None, :], (RC, G4)),
                 np.broadcast_to(bg_b[None, :], (RC, G4))], axis=0
            )
            m["bgbc"] = np.ascontiguousarray(bg.astype(np.float32))
        if with_b2:
            m["b2bc"] = np.ascontiguousarray(
                np.broadcast_to(b2[None, :], (RC, V)).astype(np.float32)
            )
        in_maps.append(m)
    return nc, in_maps


def kernel(**inputs):
    from concourse.bass_utils import run_bass_kernel_spmd

    nc, in_maps = prepare(inputs)
    res = run_bass_kernel_spmd(
        nc, in_maps, core_ids=list(range(NCORES)),
        trace=bool(int(os.environ.get("KERNEL_TRACE", "0"))),
    )
    out = np.concatenate(
        [res.results[c]["out"] for c in range(NCORES)], axis=0
    )
    kernel.last_results = res
    return out.astype(np.float32)


# revision 21
# speedup vs baseline: 1.0574x; 1.0574x over previous
# Bidirectional LSTM (B=512, T=256, E=256, U=512) + MLP + softmax(V=10000)
# on 8 trn2 NeuronCores.
#
# Distribution (v2, collective-free): each core owns 64 batch rows and runs
# BOTH directions for those rows. Within a core, partitions 0-63 carry the
# forward chain and partitions 64-127 the backward chain (time-reversed
# token stream, interleaved in the gather index table). All matmuls are
# emitted as column-tiled pairs (tile_position (0,0) / (0,64)) so the fw and
# bw halves use disjoint PE column groups and can run concurrently. The MLP
# head needs h_fw[r] (+) h_bw[r] of the same row r, which now live on the
# same core: transpose the partial product, add the partition halves in the
# free dimension, relu, then h @ W2 + softmax. No cross-core communication
# (the pairwise AllReduce of v1 cost ~31 ms of wall overhead per call).
#
# Per step t (one core):
#   gates[128,2048] (PSUM, fp32): x_t @ Wx + h_{t-1} @ Wh as lhsT.T @ rhs
#     with data transposed as stationary (xT via transposing dma_gather;
#     hT via per-step PE transpose) and bf16 weights streaming; fw rows use
#     Wx_fw/Wh_fw (cols 0-63 of the stationary), bw rows Wx_bw/Wh_bw.
#   i,f,o = sigmoid(gates[:,0:1536]); g = tanh(gates[:,1536:2048])  (ScalarE,
#     gate columns pre-permuted to [i f o g] on the host)
#   c = f*c + i*g (DVE, fp32 state);  h = o * tanh(c)  (bf16)
import os
import numpy as np
import ml_dtypes

B, T, E, U, V = 512, 256, 256, 512, 10000
G4 = 4 * U
NCORES = 8
RC = 64               # batch rows per core (each run in both directions)
PC = 2 * RC           # partition rows per core = 64 fw + 64 bw
NK_X = E // 128       # 2 contraction tiles for x
NK_H = U // 128       # 4 contraction tiles for h
NBW = 512             # matmul n-block width (one PSUM bank)
NB = G4 // NBW        # 4 n-blocks
TOK = PC * T          # 32768 tokens gathered per core
T_STEPS = int(os.environ.get("KERNEL_T", T))
CHUNK_STEPS = 4       # 512 tokens per dma_gather (>512 idxs crashes SWDGE)
CHUNK_TOK = PC * CHUNK_STEPS
NCHUNK = (T_STEPS + CHUNK_STEPS - 1) // CHUNK_STEPS
VCH = 500             # logits chunk width
NVCH = V // VCH

_prog_cache = {}


def _build_program(with_gate_bias: bool, with_b2: bool):
    import concourse.bass as bass
    import concourse.mybir as mybir
    import concourse.tile as tile
    from concourse import bacc
    from concourse.masks import make_identity
    from contextlib import ExitStack

    f32 = mybir.dt.float32
    bf16 = mybir.dt.bfloat16
    i16 = mybir.dt.int16
    AF = mybir.ActivationFunctionType

    nc = bacc.Bacc("TRN2", debug=False, enable_asserts=False, num_devices=NCORES)

    emb_d = nc.dram_tensor("emb16", [V, E], bf16, kind="ExternalInput").ap()
    idx_d = nc.dram_tensor("idx16", [128, TOK // 16], i16, kind="ExternalInput").ap()
    # dir-major: [fw k-tiles..., bw k-tiles...]
    wx_d = nc.dram_tensor("wx", [2 * NK_X, 128, G4], bf16, kind="ExternalInput").ap()
    wh_d = nc.dram_tensor("wh", [2 * NK_H, 128, G4], bf16, kind="ExternalInput").ap()
    w1_d = nc.dram_tensor("w1h", [2 * NK_H, 128, 64], bf16, kind="ExternalInput").ap()
    w2_d = nc.dram_tensor("w2", [64, V], bf16, kind="ExternalInput").ap()
    b1_d = nc.dram_tensor("b1p", [64, 1], f32, kind="ExternalInput").ap()
    if with_gate_bias:
        bg_d = nc.dram_tensor("bgbc", [128, G4], f32, kind="ExternalInput").ap()
    if with_b2:
        b2_d = nc.dram_tensor("b2bc", [64, V], f32, kind="ExternalInput").ap()
    out_d = nc.dram_tensor("out", [RC, V], f32, kind="ExternalOutput").ap()

    with tile.TileContext(nc) as tc, ExitStack() as ctx:
        const = ctx.enter_context(tc.tile_pool(name="const", bufs=1))
        gpool = ctx.enter_context(tc.tile_pool(name="gather", bufs=3))
        work = ctx.enter_context(tc.tile_pool(name="work", bufs=2))
        psum = ctx.enter_context(tc.tile_pool(name="psum", bufs=1, space="PSUM"))

        # token indices first so the first gather isn't queued behind 7MB
        # of weight DMAs
        idx_sb = const.tile([128, TOK // 16], i16)
        nc.sync.dma_start(idx_sb[:], idx_d[:])
        wx_sb = const.tile([128, 2 * NK_X, G4], bf16)
        for k in range(2 * NK_X):
            nc.sync.dma_start(wx_sb[:, k, :], wx_d[k])
        wh_sb = const.tile([128, 2 * NK_H, G4], bf16)
        for k in range(2 * NK_H):
            nc.sync.dma_start(wh_sb[:, k, :], wh_d[k])
        w1_sb = const.tile([128, 2 * NK_H, 64], bf16)
        for k in range(2 * NK_H):
            nc.sync.dma_start(w1_sb[:, k, :], w1_d[k])
        w2_sb = const.tile([64, V], bf16)
        nc.sync.dma_start(w2_sb[:], w2_d[:])
        b1_sb = const.tile([64, 1], f32)
        nc.sync.dma_start(b1_sb[:], b1_d[:])
        if with_gate_bias:
            bg_sb = const.tile([128, G4], f32)
            nc.sync.dma_start(bg_sb[:], bg_d[:])
            # DVE pre-copy so downstream tensor_tensor ops have a same-engine
            # dep (walrus TT format has a single sync-wait slot).
            bgc = const.tile([128, G4], f32)
            nc.vector.tensor_copy(bgc[:], bg_sb[:])
        if with_b2:
            b2_sb = const.tile([64, V], f32)
            nc.sync.dma_start(b2_sb[:], b2_d[:])
        ident = const.tile([128, 128], bf16)
        make_identity(nc, ident[:])
        # c state kept as two half tiles so the a-half update chain never
        # waits on the b-half (tile-granularity dep tracking)
        U2 = U // 2
        c_a = const.tile([128, U2], f32)
        c_b = const.tile([128, U2], f32)

        xg_tiles = {}

        def issue_gather(ci):
            xg = gpool.tile(
                [128, NK_X, CHUNK_TOK], bf16, tag="xg", name=f"xg{ci}"
            )
            nc.gpsimd.dma_gather(
                xg[:],
                emb_d[:],
                idx_sb[:, ci * (CHUNK_TOK // 16):(ci + 1) * (CHUNK_TOK // 16)],
                CHUNK_TOK,
                CHUNK_TOK,
                E,
                transpose=True,
            )
            xg_tiles[ci] = xg

        issue_gather(0)

        def mm_pair(out, lhsT_fw, rhs_fw, lhsT_bw, rhs_bw, start, stop):
            nc.tensor.matmul(
                out[0:RC, :], lhsT=lhsT_fw, rhs=rhs_fw,
                start=start, stop=stop, tile_position=(0, 0),
            )
            nc.tensor.matmul(
                out[RC:PC, :], lhsT=lhsT_bw, rhs=rhs_bw,
                start=start, stop=stop, tile_position=(0, RC),
            )

        hT_prev = None
        for t in range(T_STEPS):
            ci = t // CHUNK_STEPS
            w = t % CHUNK_STEPS
            if w == 1 and ci + 1 < NCHUNK:
                issue_gather(ci + 1)
            xg = xg_tiles[ci]

            # separate PSUM tiles per activation-read region (deps are
            # tracked per tile, so one tile per reader avoids false WARs
            # serializing the matmul stream)
            gif = psum.tile([128, 2 * U], f32, tag="gif", name=f"gif{t}")
            ggp = psum.tile([128, U], f32, tag="ggp", name=f"ggp{t}")
            gop = psum.tile([128, U], f32, tag="gop", name=f"gop{t}")
            n_kt = NK_X + (NK_H if hT_prev is not None else 0)

            def emit_block(out, c0, cw):
                # full contraction for one gate column range [c0, c0+cw)
                # (own PSUM bank, own accumulation group)
                ki = 0
                for k in range(NK_X):
                    mm_pair(
                        out,
                        xg[:, k, w * PC:w * PC + RC],
                        wx_sb[:, k, c0:c0 + cw],
                        xg[:, k, w * PC + RC:(w + 1) * PC],
                        wx_sb[:, NK_X + k, c0:c0 + cw],
                        start=(ki == 0),
                        stop=(ki == n_kt - 1),
                    )
                    ki += 1
                if hT_prev is not None:
                    for k in range(NK_H):
                        hT = hT_prev[k // 2]
                        kl = k % 2
                        mm_pair(
                            out,
                            hT[:, kl * 128:kl * 128 + RC],
                            wh_sb[:, k, c0:c0 + cw],
                            hT[:, kl * 128 + RC:(kl + 1) * 128],
                            wh_sb[:, NK_H + k, c0:c0 + cw],
                            start=(ki == 0),
                            stop=(ki == n_kt - 1),
                        )
                        ki += 1

            # col layout is [i f o g]; process i,f first (one sigmoid),
            # then g, then o so the c-update chain starts early
            emit_block(gif[:, 0:U], 0, U)
            emit_block(gif[:, U:2 * U], U, U)
            if with_gate_bias:
                nc.vector.tensor_add(gif[:], gif[:], bgc[:, 0:2 * U])
            ifo = work.tile([128, 2 * U], bf16, tag="ifo", name=f"ifo{t}")
            nc.scalar.activation(ifo[:], gif[:], AF.Sigmoid)
            emit_block(ggp[:], 3 * U, U)
            if with_gate_bias:
                nc.vector.tensor_add(ggp[:], ggp[:], bgc[:, 3 * U:G4])
            gga = work.tile([128, U2], bf16, tag="gga", name=f"gga{t}")
            nc.scalar.activation(gga[:], ggp[:, 0:U2], AF.Tanh)
            ggb = work.tile([128, U2], bf16, tag="ggb", name=f"ggb{t}")
            nc.scalar.activation(ggb[:], ggp[:, U2:U], AF.Tanh)
            emit_block(gop[:], 2 * U, U)
            if with_gate_bias:
                nc.vector.tensor_add(gop[:], gop[:], bgc[:, 2 * U:3 * U])
            oo = work.tile([128, U], bf16, tag="oo", name=f"oo{t}")
            nc.scalar.activation(oo[:], gop[:], AF.Sigmoid)

            # c/h updated in independent a/b halves (256 cols each) so the
            # a-half's hT lands early and next step's k0/k1 matmuls start
            # while the b-half still runs
            if t == 0:
                nc.vector.tensor_mul(c_a[:], ifo[:, 0:U2], gga[:])
                nc.vector.tensor_mul(c_b[:], ifo[:, U2:U], ggb[:])
            else:
                fca = work.tile([128, U2], f32, tag="fca", name=f"fca{t}")
                nc.vector.tensor_mul(fca[:], ifo[:, U:U + U2], c_a[:])
                ppa = work.tile([128, U2], bf16, tag="ppa", name=f"ppa{t}")
                nc.vector.tensor_mul(ppa[:], ifo[:, 0:U2], gga[:])
                nc.vector.tensor_add(c_a[:], fca[:], ppa[:])
                fcb = work.tile([128, U2], f32, tag="fcb", name=f"fcb{t}")
                nc.vector.tensor_mul(fcb[:], ifo[:, U + U2:2 * U], c_b[:])
                ppb = work.tile([128, U2], bf16, tag="ppb", name=f"ppb{t}")
                nc.vector.tensor_mul(ppb[:], ifo[:, U2:U], ggb[:])
                nc.vector.tensor_add(c_b[:], fcb[:], ppb[:])
            tcta = work.tile([128, U2], bf16, tag="tcta", name=f"tcta{t}")
            nc.scalar.activation(tcta[:], c_a[:], AF.Tanh)
            tctb = work.tile([128, U2], bf16, tag="tctb", name=f"tctb{t}")
            nc.scalar.activation(tctb[:], c_b[:], AF.Tanh)
            ha = work.tile([128, U2], bf16, tag="ha", name=f"ha{t}")
            nc.vector.tensor_mul(ha[:], oo[:, 0:U2], tcta[:])
            hb = work.tile([128, U2], bf16, tag="hb", name=f"hb{t}")
            nc.vector.tensor_mul(hb[:], oo[:, U2:U], tctb[:])

            trpa = psum.tile([128, U2], bf16, tag="trpa", bufs=2, name=f"trpa{t}")
            for k in range(2):
                nc.tensor.transpose(
                    trpa[:, k * 128:(k + 1) * 128],
                    ha[:, k * 128:(k + 1) * 128],
                    ident[:],
                )
            hTa = work.tile([128, U2], bf16, tag="hTa", name=f"hTa{t}")
            nc.vector.tensor_copy(hTa[:], trpa[:])
            trpb = psum.tile([128, U2], bf16, tag="trpb", bufs=2, name=f"trpb{t}")
            for k in range(2):
                nc.tensor.transpose(
                    trpb[:, k * 128:(k + 1) * 128],
                    hb[:, k * 128:(k + 1) * 128],
                    ident[:],
                )
            hTb = work.tile([128, U2], bf16, tag="hTb", name=f"hTb{t}")
            nc.vector.tensor_copy(hTb[:], trpb[:])
            hT_prev = (hTa, hTb)

        # ---- MLP head: P[r] = h_fw[r] @ W1a (rows 0-63) / h_bw[r] @ W1b
        # (rows 64-127); transpose, add the halves, relu(+b1) -> hidT.
        pps = psum.tile([128, 64], f32, tag="gif", name="pps")
        for k in range(NK_H):
            hTk = hT_prev[k // 2]
            kl = k % 2
            nc.tensor.matmul(
                pps[0:RC, :], lhsT=hTk[:, kl * 128:kl * 128 + RC],
                rhs=w1_sb[:, k, :],
                start=(k == 0), stop=(k == NK_H - 1), tile_position=(0, 0),
            )
            nc.tensor.matmul(
                pps[RC:PC, :], lhsT=hTk[:, kl * 128 + RC:(kl + 1) * 128],
                rhs=w1_sb[:, NK_H + k, :],
                start=(k == 0), stop=(k == NK_H - 1), tile_position=(0, RC),
            )
        p_sb = work.tile([128, 64], bf16, tag="p_sb", bufs=1)
        nc.vector.tensor_copy(p_sb[:], pps[:])
        ppt = psum.tile([64, 128], bf16, tag="trpa", bufs=2, name="ppt")
        nc.tensor.transpose(ppt[:], p_sb[:], ident[:])
        pt_sb = work.tile([64, 128], bf16, tag="pt_sb", bufs=1)
        nc.vector.tensor_copy(pt_sb[:], ppt[:])
        psum_h = work.tile([64, 64], bf16, tag="psum_h", bufs=1)
        nc.vector.tensor_add(psum_h[:], pt_sb[:, 0:RC], pt_sb[:, RC:PC])
        hidT = work.tile([64, 64], bf16, tag="hidT", bufs=1)
        nc.scalar.activation(hidT[:], psum_h[:], AF.Relu, bias=b1_sb[:])

        logits = work.tile([64, V], f32, tag="logits", bufs=1)
        for vc in range(NVCH):
            lp = psum.tile([64, VCH], f32, tag="trpa", bufs=2, name=f"lp{vc}")
            nc.tensor.matmul(
                lp[:],
                lhsT=hidT[:],
                rhs=w2_sb[:, vc * VCH:(vc + 1) * VCH],
                start=True,
                stop=True,
            )
            nc.vector.tensor_copy(logits[:, vc * VCH:(vc + 1) * VCH], lp[:])
        if with_b2:
            nc.vector.tensor_add(logits[:], logits[:], b2_sb[:])

        negmax = work.tile([64, 1], f32, tag="negmax", bufs=1)
        nc.vector.reduce_max(
            negmax[:], logits[:], axis=mybir.AxisListType.X, negate=True
        )
        exps = work.tile([64, V], bf16, tag="exps", bufs=1)
        sume = work.tile([64, 1], f32, tag="sume", bufs=1)
        nc.scalar.activation(
            exps[:], logits[:], AF.Exp, bias=negmax[:], accum_out=sume[:]
        )
        rcp = work.tile([64, 1], f32, tag="rcp", bufs=1)
        nc.vector.reciprocal(rcp[:], sume[:])
        nc.vector.tensor_scalar_mul(logits[:], exps[:], rcp[:])
        nc.sync.dma_start(out_d[:], logits[:])

    nc.finalize()
    return nc


def _get_program(with_gate_bias: bool, with_b2: bool):
    key = (with_gate_bias, with_b2, T_STEPS)
    if key not in _prog_cache:
        _prog_cache[key] = _build_program(with_gate_bias, with_b2)
    return _prog_cache[key]


# gate column permutation: reference order [i f g o] -> kernel order [i f o g]
_PERM = np.concatenate(
    [np.arange(0, U), np.arange(U, 2 * U), np.arange(3 * U, 4 * U),
     np.arange(2 * U, 3 * U)]
)


def _pack_w(Wx, Wh, b):
    bf = ml_dtypes.bfloat16
    wxp = np.ascontiguousarray(
        Wx[:, _PERM].reshape(NK_X, 128, G4).astype(bf)
    )
    whp = np.ascontiguousarray(
        Wh[:, _PERM].reshape(NK_H, 128, G4).astype(bf)
    )
    bp = np.ascontiguousarray(b[_PERM].astype(np.float32))
    return wxp, whp, bp


def _make_idx(tokens_tmajor_flat):
    # dma_gather reads index i from [i % 16, i // 16]; the 16-partition index
    # block must be replicated for each of the 8 gpsimd cores (128 partitions).
    wrapped = tokens_tmajor_flat.astype(np.int16).reshape(-1, 16).T
    return np.ascontiguousarray(np.tile(wrapped, (8, 1)))


def prepare(inputs):
    """Build (nc, in_maps) for the 8 cores from full unsharded inputs."""
    bf = ml_dtypes.bfloat16
    sentence = np.asarray(inputs["sentence"])
    emb = np.asarray(inputs["emb"], np.float32)
    Wx_fw = np.asarray(inputs["Wx_fw"], np.float32)
    Wh_fw = np.asarray(inputs["Wh_fw"], np.float32)
    b_fw = np.asarray(inputs["b_fw"], np.float32)
    Wx_bw = np.asarray(inputs["Wx_bw"], np.float32)
    Wh_bw = np.asarray(inputs["Wh_bw"], np.float32)
    b_bw = np.asarray(inputs["b_bw"], np.float32)
    W1 = np.asarray(inputs["W1"], np.float32)
    b1 = np.asarray(inputs["b1"], np.float32)
    W2 = np.asarray(inputs["W2"], np.float32)
    b2 = np.asarray(inputs["b2"], np.float32)

    with_gate_bias = bool(np.any(b_fw) or np.any(b_bw))
    with_b2 = bool(np.any(b2))
    nc = _get_program(with_gate_bias, with_b2)

    emb16 = np.ascontiguousarray(emb.astype(bf))
    wx_f, wh_f, bg_f = _pack_w(Wx_fw, Wh_fw, b_fw)
    wx_b, wh_b, bg_b = _pack_w(Wx_bw, Wh_bw, b_bw)
    wx = np.ascontiguousarray(np.concatenate([wx_f, wx_b], axis=0))
    wh = np.ascontiguousarray(np.concatenate([wh_f, wh_b], axis=0))
    w1f = W1[0:U].reshape(NK_H, 128, 64)
    w1b = W1[U:2 * U].reshape(NK_H, 128, 64)
    w1 = np.ascontiguousarray(
        np.concatenate([w1f, w1b], axis=0).astype(bf)
    )
    w2p = np.ascontiguousarray(W2.astype(bf))
    b1p = np.ascontiguousarray(b1.reshape(64, 1).astype(np.float32))

    in_maps = []
    for c in range(NCORES):
        rows = slice(RC * c, RC * c + RC)
        toks_fw = sentence[rows][:, :T]
        toks_bw = toks_fw[:, ::-1]
        # t-major, per step [64 fw tokens | 64 bw tokens]
        flat = np.ascontiguousarray(
            np.concatenate([toks_fw.T, toks_bw.T], axis=1)
        ).reshape(-1)
        m = {
            "emb16": emb16,
            "idx16": _make_idx(flat),
            "wx": wx,
            "wh": wh,
            "w1h": w1,
            "w2": w2p,
            "b1p": b1p,
        }
        if with_gate_bias:
            bg = np.concatenate(
                [np.broadcast_to(bg_f[None, :], (RC, G4)),
                 np.broadcast_to(bg_b[None, :], (RC, G4))], axis=0
            )
            m["bgbc"] = np.ascontiguousarray(bg.astype(np.float32))
        if with_b2:
            m["b2bc"] = np.ascontiguousarray(
                np.broadcast_to(b2[None, :], (RC, V)).astype(np.float32)
            )
        in_maps.append(m)
    return nc, in_maps


def kernel(**inputs):
    from concourse.bass_utils import run_bass_kernel_spmd

    nc, in_maps = prepare(inputs)
    res = run_bass_kernel_spmd(
        nc, in_maps, core_ids=list(range(NCORES)),
        trace=bool(int(os.environ.get("KERNEL_TRACE", "0"))),
    )
    out = np.concatenate(
        [res.results[c]["out"] for c in range(NCORES)], axis=0
    )
    kernel.last_results = res
    return out.astype(np.float32)


# revision 22
# speedup vs baseline: 1.0594x; 1.0018x over previous
# Bidirectional LSTM (B=512, T=256, E=256, U=512) + MLP + softmax(V=10000)
# on 8 trn2 NeuronCores.
#
# Distribution (v2, collective-free): each core owns 64 batch rows and runs
# BOTH directions for those rows. Within a core, partitions 0-63 carry the
# forward chain and partitions 64-127 the backward chain (time-reversed
# token stream, interleaved in the gather index table). All matmuls are
# emitted as column-tiled pairs (tile_position (0,0) / (0,64)) so the fw and
# bw halves use disjoint PE column groups and can run concurrently. The MLP
# head needs h_fw[r] (+) h_bw[r] of the same row r, which now live on the
# same core: transpose the partial product, add the partition halves in the
# free dimension, relu, then h @ W2 + softmax. No cross-core communication
# (the pairwise AllReduce of v1 cost ~31 ms of wall overhead per call).
#
# Per step t (one core):
#   gates[128,2048] (PSUM, fp32): x_t @ Wx + h_{t-1} @ Wh as lhsT.T @ rhs
#     with data transposed as stationary (xT via transposing dma_gather;
#     hT via per-step PE transpose) and bf16 weights streaming; fw rows use
#     Wx_fw/Wh_fw (cols 0-63 of the stationary), bw rows Wx_bw/Wh_bw.
#   i,f,o = sigmoid(gates[:,0:1536]); g = tanh(gates[:,1536:2048])  (ScalarE,
#     gate columns pre-permuted to [i f o g] on the host)
#   c = f*c + i*g (DVE, fp32 state);  h = o * tanh(c)  (bf16)
import os
import numpy as np
import ml_dtypes

B, T, E, U, V = 512, 256, 256, 512, 10000
G4 = 4 * U
NCORES = 8
RC = 64               # batch rows per core (each run in both directions)
PC = 2 * RC           # partition rows per core = 64 fw + 64 bw
NK_X = E // 128       # 2 contraction tiles for x
NK_H = U // 128       # 4 contraction tiles for h
NBW = 512             # matmul n-block width (one PSUM bank)
NB = G4 // NBW        # 4 n-blocks
TOK = PC * T          # 32768 tokens gathered per core
T_STEPS = int(os.environ.get("KERNEL_T", T))
CHUNK_STEPS = 4       # 512 tokens per dma_gather (>512 idxs crashes SWDGE)
CHUNK_TOK = PC * CHUNK_STEPS
NCHUNK = (T_STEPS + CHUNK_STEPS - 1) // CHUNK_STEPS
VCH = 500             # logits chunk width
NVCH = V // VCH

_prog_cache = {}


def _build_program(with_gate_bias: bool, with_b2: bool):
    import concourse.bass as bass
    import concourse.mybir as mybir
    import concourse.tile as tile
    from concourse import bacc
    from concourse.masks import make_identity
    from contextlib import ExitStack

    f32 = mybir.dt.float32
    bf16 = mybir.dt.bfloat16
    i16 = mybir.dt.int16
    AF = mybir.ActivationFunctionType

    nc = bacc.Bacc("TRN2", debug=False, enable_asserts=False, num_devices=NCORES)

    emb_d = nc.dram_tensor("emb16", [V, E], bf16, kind="ExternalInput").ap()
    idx_d = nc.dram_tensor("idx16", [128, TOK // 16], i16, kind="ExternalInput").ap()
    # dir-major: [fw k-tiles..., bw k-tiles...]
    wx_d = nc.dram_tensor("wx", [2 * NK_X, 128, G4], bf16, kind="ExternalInput").ap()
    wh_d = nc.dram_tensor("wh", [2 * NK_H, 128, G4], bf16, kind="ExternalInput").ap()
    w1_d = nc.dram_tensor("w1h", [2 * NK_H, 128, 64], bf16, kind="ExternalInput").ap()
    w2_d = nc.dram_tensor("w2", [64, V], bf16, kind="ExternalInput").ap()
    b1_d = nc.dram_tensor("b1p", [64, 1], f32, kind="ExternalInput").ap()
    if with_gate_bias:
        bg_d = nc.dram_tensor("bgbc", [128, G4], f32, kind="ExternalInput").ap()
    if with_b2:
        b2_d = nc.dram_tensor("b2bc", [64, V], f32, kind="ExternalInput").ap()
    out_d = nc.dram_tensor("out", [RC, V], f32, kind="ExternalOutput").ap()

    with tile.TileContext(nc) as tc, ExitStack() as ctx:
        const = ctx.enter_context(tc.tile_pool(name="const", bufs=1))
        gpool = ctx.enter_context(tc.tile_pool(name="gather", bufs=3))
        work = ctx.enter_context(tc.tile_pool(name="work", bufs=3))
        psum = ctx.enter_context(tc.tile_pool(name="psum", bufs=1, space="PSUM"))

        # token indices first so the first gather isn't queued behind 7MB
        # of weight DMAs
        idx_sb = const.tile([128, TOK // 16], i16)
        nc.sync.dma_start(idx_sb[:], idx_d[:])
        wx_sb = const.tile([128, 2 * NK_X, G4], bf16)
        for k in range(2 * NK_X):
            nc.sync.dma_start(wx_sb[:, k, :], wx_d[k])
        wh_sb = const.tile([128, 2 * NK_H, G4], bf16)
        for k in range(2 * NK_H):
            nc.sync.dma_start(wh_sb[:, k, :], wh_d[k])
        w1_sb = const.tile([128, 2 * NK_H, 64], bf16)
        for k in range(2 * NK_H):
            nc.sync.dma_start(w1_sb[:, k, :], w1_d[k])
        w2_sb = const.tile([64, V], bf16)
        nc.sync.dma_start(w2_sb[:], w2_d[:])
        b1_sb = const.tile([64, 1], f32)
        nc.sync.dma_start(b1_sb[:], b1_d[:])
        if with_gate_bias:
            bg_sb = const.tile([128, G4], f32)
            nc.sync.dma_start(bg_sb[:], bg_d[:])
            # DVE pre-copy so downstream tensor_tensor ops have a same-engine
            # dep (walrus TT format has a single sync-wait slot).
            bgc = const.tile([128, G4], f32)
            nc.vector.tensor_copy(bgc[:], bg_sb[:])
        if with_b2:
            b2_sb = const.tile([64, V], f32)
            nc.sync.dma_start(b2_sb[:], b2_d[:])
        ident = const.tile([128, 128], bf16)
        make_identity(nc, ident[:])
        # c state kept as two half tiles so the a-half update chain never
        # waits on the b-half (tile-granularity dep tracking)
        U2 = U // 2
        c_a = const.tile([128, U2], f32)
        c_b = const.tile([128, U2], f32)

        xg_tiles = {}

        def issue_gather(ci):
            xg = gpool.tile(
                [128, NK_X, CHUNK_TOK], bf16, tag="xg", name=f"xg{ci}"
            )
            nc.gpsimd.dma_gather(
                xg[:],
                emb_d[:],
                idx_sb[:, ci * (CHUNK_TOK // 16):(ci + 1) * (CHUNK_TOK // 16)],
                CHUNK_TOK,
                CHUNK_TOK,
                E,
                transpose=True,
            )
            xg_tiles[ci] = xg

        issue_gather(0)

        def mm_pair(out, lhsT_fw, rhs_fw, lhsT_bw, rhs_bw, start, stop):
            nc.tensor.matmul(
                out[0:RC, :], lhsT=lhsT_fw, rhs=rhs_fw,
                start=start, stop=stop, tile_position=(0, 0),
            )
            nc.tensor.matmul(
                out[RC:PC, :], lhsT=lhsT_bw, rhs=rhs_bw,
                start=start, stop=stop, tile_position=(0, RC),
            )

        hT_prev = None
        for t in range(T_STEPS):
            ci = t // CHUNK_STEPS
            w = t % CHUNK_STEPS
            if w == 1 and ci + 1 < NCHUNK:
                issue_gather(ci + 1)
            xg = xg_tiles[ci]

            # separate PSUM tiles per activation-read region (deps are
            # tracked per tile, so one tile per reader avoids false WARs
            # serializing the matmul stream)
            gif = psum.tile([128, 2 * U], f32, tag="gif", name=f"gif{t}")
            ggp = psum.tile([128, U], f32, tag="ggp", name=f"ggp{t}")
            gop = psum.tile([128, U], f32, tag="gop", name=f"gop{t}")
            n_kt = NK_X + (NK_H if hT_prev is not None else 0)

            def emit_block(out, c0, cw):
                # full contraction for one gate column range [c0, c0+cw)
                # (own PSUM bank, own accumulation group)
                ki = 0
                for k in range(NK_X):
                    mm_pair(
                        out,
                        xg[:, k, w * PC:w * PC + RC],
                        wx_sb[:, k, c0:c0 + cw],
                        xg[:, k, w * PC + RC:(w + 1) * PC],
                        wx_sb[:, NK_X + k, c0:c0 + cw],
                        start=(ki == 0),
                        stop=(ki == n_kt - 1),
                    )
                    ki += 1
                if hT_prev is not None:
                    for k in range(NK_H):
                        hT = hT_prev[k // 2]
                        kl = k % 2
                        mm_pair(
                            out,
                            hT[:, kl * 128:kl * 128 + RC],
                            wh_sb[:, k, c0:c0 + cw],
                            hT[:, kl * 128 + RC:(kl + 1) * 128],
                            wh_sb[:, NK_H + k, c0:c0 + cw],
                            start=(ki == 0),
                            stop=(ki == n_kt - 1),
                        )
                        ki += 1

            # col layout is [i f o g]; process i,f first (one sigmoid),
            # then g, then o so the c-update chain starts early
            emit_block(gif[:, 0:U], 0, U)
            emit_block(gif[:, U:2 * U], U, U)
            if with_gate_bias:
                nc.vector.tensor_add(gif[:], gif[:], bgc[:, 0:2 * U])
            ifo = work.tile([128, 2 * U], bf16, tag="ifo", name=f"ifo{t}")
            nc.scalar.activation(ifo[:], gif[:], AF.Sigmoid)
            emit_block(ggp[:], 3 * U, U)
            if with_gate_bias:
                nc.vector.tensor_add(ggp[:], ggp[:], bgc[:, 3 * U:G4])
            gga = work.tile([128, U2], bf16, tag="gga", name=f"gga{t}")
            nc.scalar.activation(gga[:], ggp[:, 0:U2], AF.Tanh)
            ggb = work.tile([128, U2], bf16, tag="ggb", name=f"ggb{t}")
            nc.scalar.activation(ggb[:], ggp[:, U2:U], AF.Tanh)
            emit_block(gop[:], 2 * U, U)
            if with_gate_bias:
                nc.vector.tensor_add(gop[:], gop[:], bgc[:, 2 * U:3 * U])
            oo = work.tile([128, U], bf16, tag="oo", name=f"oo{t}")
            nc.scalar.activation(oo[:], gop[:], AF.Sigmoid)

            # c/h updated in independent a/b halves (256 cols each) so the
            # a-half's hT lands early and next step's k0/k1 matmuls start
            # while the b-half still runs
            if t == 0:
                nc.vector.tensor_mul(c_a[:], ifo[:, 0:U2], gga[:])
                nc.vector.tensor_mul(c_b[:], ifo[:, U2:U], ggb[:])
            else:
                fca = work.tile([128, U2], f32, tag="fca", name=f"fca{t}")
                nc.vector.tensor_mul(fca[:], ifo[:, U:U + U2], c_a[:])
                ppa = work.tile([128, U2], bf16, tag="ppa", name=f"ppa{t}")
                nc.vector.tensor_mul(ppa[:], ifo[:, 0:U2], gga[:])
                nc.vector.tensor_add(c_a[:], fca[:], ppa[:])
                fcb = work.tile([128, U2], f32, tag="fcb", name=f"fcb{t}")
                nc.vector.tensor_mul(fcb[:], ifo[:, U + U2:2 * U], c_b[:])
                ppb = work.tile([128, U2], bf16, tag="ppb", name=f"ppb{t}")
                nc.vector.tensor_mul(ppb[:], ifo[:, U2:U], ggb[:])
                nc.vector.tensor_add(c_b[:], fcb[:], ppb[:])
            tcta = work.tile([128, U2], bf16, tag="tcta", name=f"tcta{t}")
            nc.scalar.activation(tcta[:], c_a[:], AF.Tanh)
            tctb = work.tile([128, U2], bf16, tag="tctb", name=f"tctb{t}")
            nc.scalar.activation(tctb[:], c_b[:], AF.Tanh)
            ha = work.tile([128, U2], bf16, tag="ha", name=f"ha{t}")
            nc.vector.tensor_mul(ha[:], oo[:, 0:U2], tcta[:])
            hb = work.tile([128, U2], bf16, tag="hb", name=f"hb{t}")
            nc.vector.tensor_mul(hb[:], oo[:, U2:U], tctb[:])

            trpa = psum.tile([128, U2], bf16, tag="trpa", bufs=2, name=f"trpa{t}")
            for k in range(2):
                nc.tensor.transpose(
                    trpa[:, k * 128:(k + 1) * 128],
                    ha[:, k * 128:(k + 1) * 128],
                    ident[:],
                )
            hTa = work.tile([128, U2], bf16, tag="hTa", name=f"hTa{t}")
            nc.vector.tensor_copy(hTa[:], trpa[:])
            trpb = psum.tile([128, U2], bf16, tag="trpb", bufs=2, name=f"trpb{t}")
            for k in range(2):
                nc.tensor.transpose(
                    trpb[:, k * 128:(k + 1) * 128],
                    hb[:, k * 128:(k + 1) * 128],
                    ident[:],
                )
            hTb = work.tile([128, U2], bf16, tag="hTb", name=f"hTb{t}")
            nc.vector.tensor_copy(hTb[:], trpb[:])
            hT_prev = (hTa, hTb)

        # ---- MLP head: P[r] = h_fw[r] @ W1a (rows 0-63) / h_bw[r] @ W1b
        # (rows 64-127); transpose, add the halves, relu(+b1) -> hidT.
        pps = psum.tile([128, 64], f32, tag="gif", name="pps")
        for k in range(NK_H):
            hTk = hT_prev[k // 2]
            kl = k % 2
            nc.tensor.matmul(
                pps[0:RC, :], lhsT=hTk[:, kl * 128:kl * 128 + RC],
                rhs=w1_sb[:, k, :],
                start=(k == 0), stop=(k == NK_H - 1), tile_position=(0, 0),
            )
            nc.tensor.matmul(
                pps[RC:PC, :], lhsT=hTk[:, kl * 128 + RC:(kl + 1) * 128],
                rhs=w1_sb[:, NK_H + k, :],
                start=(k == 0), stop=(k == NK_H - 1), tile_position=(0, RC),
            )
        p_sb = work.tile([128, 64], bf16, tag="p_sb", bufs=1)
        nc.vector.tensor_copy(p_sb[:], pps[:])
        ppt = psum.tile([64, 128], bf16, tag="trpa", bufs=2, name="ppt")
        nc.tensor.transpose(ppt[:], p_sb[:], ident[:])
        pt_sb = work.tile([64, 128], bf16, tag="pt_sb", bufs=1)
        nc.vector.tensor_copy(pt_sb[:], ppt[:])
        psum_h = work.tile([64, 64], bf16, tag="psum_h", bufs=1)
        nc.vector.tensor_add(psum_h[:], pt_sb[:, 0:RC], pt_sb[:, RC:PC])
        hidT = work.tile([64, 64], bf16, tag="hidT", bufs=1)
        nc.scalar.activation(hidT[:], psum_h[:], AF.Relu, bias=b1_sb[:])

        logits = work.tile([64, V], f32, tag="logits", bufs=1)
        for vc in range(NVCH):
            lp = psum.tile([64, VCH], f32, tag="trpa", bufs=2, name=f"lp{vc}")
            nc.tensor.matmul(
                lp[:],
                lhsT=hidT[:],
                rhs=w2_sb[:, vc * VCH:(vc + 1) * VCH],
                start=True,
                stop=True,
            )
            nc.vector.tensor_copy(logits[:, vc * VCH:(vc + 1) * VCH], lp[:])
        if with_b2:
            nc.vector.tensor_add(logits[:], logits[:], b2_sb[:])

        negmax = work.tile([64, 1], f32, tag="negmax", bufs=1)
        nc.vector.reduce_max(
            negmax[:], logits[:], axis=mybir.AxisListType.X, negate=True
        )
        exps = work.tile([64, V], bf16, tag="exps", bufs=1)
        sume = work.tile([64, 1], f32, tag="sume", bufs=1)
        nc.scalar.activation(
            exps[:], logits[:], AF.Exp, bias=negmax[:], accum_out=sume[:]
        )
        rcp = work.tile([64, 1], f32, tag="rcp", bufs=1)
        nc.vector.reciprocal(rcp[:], sume[:])
        nc.vector.tensor_scalar_mul(logits[:], exps[:], rcp[:])
        nc.sync.dma_start(out_d[:], logits[:])

    nc.finalize()
    return nc


def _get_program(with_gate_bias: bool, with_b2: bool):
    key = (with_gate_bias, with_b2, T_STEPS)
    if key not in _prog_cache:
        _prog_cache[key] = _build_program(with_gate_bias, with_b2)
    return _prog_cache[key]


# gate column permutation: reference order [i f g o] -> kernel order [i f o g]
_PERM = np.concatenate(
    [np.arange(0, U), np.arange(U, 2 * U), np.arange(3 * U, 4 * U),
     np.arange(2 * U, 3 * U)]
)


def _pack_w(Wx, Wh, b):
    bf = ml_dtypes.bfloat16
    wxp = np.ascontiguousarray(
        Wx[:, _PERM].reshape(NK_X, 128, G4).astype(bf)
    )
    whp = np.ascontiguousarray(
        Wh[:, _PERM].reshape(NK_H, 128, G4).astype(bf)
    )
    bp = np.ascontiguousarray(b[_PERM].astype(np.float32))
    return wxp, whp, bp


def _make_idx(tokens_tmajor_flat):
    # dma_gather reads index i from [i % 16, i // 16]; the 16-partition index
    # block must be replicated for each of the 8 gpsimd cores (128 partitions).
    wrapped = tokens_tmajor_flat.astype(np.int16).reshape(-1, 16).T
    return np.ascontiguousarray(np.tile(wrapped, (8, 1)))


def prepare(inputs):
    """Build (nc, in_maps) for the 8 cores from full unsharded inputs."""
    bf = ml_dtypes.bfloat16
    sentence = np.asarray(inputs["sentence"])
    emb = np.asarray(inputs["emb"], np.float32)
    Wx_fw = np.asarray(inputs["Wx_fw"], np.float32)
    Wh_fw = np.asarray(inputs["Wh_fw"], np.float32)
    b_fw = np.asarray(inputs["b_fw"], np.float32)
    Wx_bw = np.asarray(inputs["Wx_bw"], np.float32)
    Wh_bw = np.asarray(inputs["Wh_bw"], np.float32)
    b_bw = np.asarray(inputs["b_bw"], np.float32)
    W1 = np.asarray(inputs["W1"], np.float32)
    b1 = np.asarray(inputs["b1"], np.float32)
    W2 = np.asarray(inputs["W2"], np.float32)
    b2 = np.asarray(inputs["b2"], np.float32)

    with_gate_bias = bool(np.any(b_fw) or np.any(b_bw))
    with_b2 = bool(np.any(b2))
    nc = _get_program(with_gate_bias, with_b2)

    emb16 = np.ascontiguousarray(emb.astype(bf))
    wx_f, wh_f, bg_f = _pack_w(Wx_fw, Wh_fw, b_fw)
    wx_b, wh_b, bg_b = _pack_w(Wx_bw, Wh_bw, b_bw)
    wx = np.ascontiguousarray(np.concatenate([wx_f, wx_b], axis=0))
    wh = np.ascontiguousarray(np.concatenate([wh_f, wh_b], axis=0))
    w1f = W1[0:U].reshape(NK_H, 128, 64)
    w1b = W1[U:2 * U].reshape(NK_H, 128, 64)
    w1 = np.ascontiguousarray(
        np.concatenate([w1f, w1b], axis=0).astype(bf)
    )
    w2p = np.ascontiguousarray(W2.astype(bf))
    b1p = np.ascontiguousarray(b1.reshape(64, 1).astype(np.float32))

    in_maps = []
    for c in range(NCORES):
        rows = slice(RC * c, RC * c + RC)
        toks_fw = sentence[rows][:, :T]
        toks_bw = toks_fw[:, ::-1]
        # t-major, per step [64 fw tokens | 64 bw tokens]
        flat = np.ascontiguousarray(
            np.concatenate([toks_fw.T, toks_bw.T], axis=1)
        ).reshape(-1)
        m = {
            "emb16": emb16,
            "idx16": _make_idx(flat),
            "wx": wx,
            "wh": wh,
            "w1h": w1,
            "w2": w2p,
            "b1p": b1p,
        }
        if with_gate_bias:
            bg = np.concatenate(
                [np.broadcast_to(bg_f[None, :], (RC, G4)),
                 np.broadcast_to(bg_b[None, :], (RC, G4))], axis=0
            )
            m["bgbc"] = np.ascontiguousarray(bg.astype(np.float32))
        if with_b2:
            m["b2bc"] = np.ascontiguousarray(
                np.broadcast_to(b2[None, :], (RC, V)).astype(np.float32)
            )
        in_maps.append(m)
    return nc, in_maps


def kernel(**inputs):
    from concourse.bass_utils import run_bass_kernel_spmd

    nc, in_maps = prepare(inputs)
    res = run_bass_kernel_spmd(
        nc, in_maps, core_ids=list(range(NCORES)),
        trace=bool(int(os.environ.get("KERNEL_TRACE", "0"))),
    )
    out = np.concatenate(
        [res.results[c]["out"] for c in range(NCORES)], axis=0
    )
    kernel.last_results = res
    return out.astype(np.float32)
